# revision 1
# baseline (speedup 1.0000x reference)
"""Bass/Tile kernel for nn_EnergyDipolesMACE on 8 TRN2 NeuronCores.

Host (index-only prep): sort edges by destination, shard destination nodes
across cores (1024 each, 8 windows of 128), pad each window's edge list to a
multiple of 128 (cap 18 chunks), format int16 gather indices.

Device per core: gather source-node rows (pos + h scalar) from a DRAM table
via SWDGE dma_gather (4 queues); geometry (Y, radial basis) once; per
iteration: radial MLP in f32r on PE (features on partitions), per-chunk
message tensor-product on DVE, scatter-sum via PE matmuls with device-built
bf16 one-hot blocks (streamed through DRAM), node-level update in f32,
AllGather of the updated scalar channel between iterations.
"""
import math
import numpy as np

import concourse.bacc as bacc
import concourse.bass as bass
import concourse.tile as tile
from concourse import mybir

# allow 128B gather payloads (row stride stays 256B-aligned; probed on HW)
import textwrap as _tw, inspect as _ins
_gsrc = _tw.dedent(_ins.getsource(bass.BassGpSimd.dma_gather))
if "% 256 == 0" in _gsrc:
    _gsrc = _gsrc.replace("elem_size_bytes > 0 and elem_size_bytes % 256 == 0",
                          "elem_size_bytes > 0 and elem_size_bytes % 128 == 0")
    _gns = dict(bass.__dict__)
    exec(compile(_gsrc, "<patched_dma_gather>", "exec"), _gns)
    bass.BassGpSimd.dma_gather = _gns["dma_gather"]

f32 = mybir.dt.float32
f32r = mybir.dt.float32r
bf16 = mybir.dt.bfloat16
i16 = mybir.dt.int16
AF = mybir.ActivationFunctionType
ALU = mybir.AluOpType

N, E, C, Z, G, NB, NSH = 8192, 131072, 32, 10, 16, 8, 9
R_MAX, P_CUT, AVG_NEIGH = 5.0, 5, 16.0
LMAP = np.array([0, 1, 1, 1, 2, 2, 2, 2, 2])
NCORES = 8
NPC = N // NCORES
WIN = 128
WPC = NPC // WIN               # 8 windows/core
W_CAP = 18                     # chunks per window
CHUNK = 128
L_PAD = WPC * W_CAP * CHUNK    # 18432
NCHUNKS = L_PAD // CHUNK       # 144
SC = 384                       # MLP superchunk (3 chunks; 6 per window)
NSC = L_PAD // SC              # 48
NQ = 4                         # SWDGE queues
IDX_COLS = L_PAD // 16         # 1152
TROW = 64                      # table row f32 elems (256B)
MC = NSH * C                   # 288
S3, S5, S15 = 3.0 ** 0.5, 5.0 ** 0.5, 15.0 ** 0.5
PREF = (2.0 / R_MAX) ** 0.5
PCF = float(P_CUT)
ENV_A = -(PCF + 1.0) * (PCF + 2.0) / 2.0
ENV_B = PCF * (PCF + 2.0)
ENV_C = -PCF * (PCF + 1.0) / 2.0
TWO_PI = 2 * math.pi


def host_prep(inputs):
    snd = np.asarray(inputs["edge_index"])[0].astype(np.int64)
    rcv = np.asarray(inputs["edge_index"])[1].astype(np.int64)
    batch = np.asarray(inputs["batch"]).astype(np.int64)
    positions = np.asarray(inputs["positions"], np.float32)
    node_attrs = np.asarray(inputs["node_attrs"], np.float32)
    charges = np.asarray(inputs["charges"], np.float32)

    order = np.argsort(rcv, kind="stable")
    snd_s, rcv_s = snd[order], rcv[order]
    win_id = rcv_s // WIN
    counts = np.bincount(win_id, minlength=N // WIN)
    assert counts.max() <= W_CAP * CHUNK, f"window overflow: {counts.max()}"

    naT = np.ascontiguousarray(node_attrs.T)
    pos4 = np.concatenate([positions, np.zeros((N, 1), np.float32)], 1)
    iota = np.tile(np.arange(128, dtype=np.float32)[None, :], (128, 1))
    ident = np.eye(128, dtype=np.float32)
    nvec = np.tile((np.arange(1, NB + 1, dtype=np.float32) * math.pi / R_MAX)[None, :],
                   (128, 1))
    R0s = np.stack([np.asarray(inputs["R0"][i], np.float32) for i in range(2)], 1)
    R1s = np.stack([np.asarray(inputs["R1"][i], np.float32) for i in range(2)], 1)
    R2s = np.stack([np.asarray(inputs["R2"][i], np.float32) for i in range(2)], 1)
    R3e = np.stack([np.asarray(inputs["R3"][i], np.float32)
                    .reshape(64, 3, C)[:, LMAP, :].reshape(64, MC)
                    for i in range(2)], 1)
    Wmix = np.stack([np.asarray(inputs["W_mix"][i], np.float32)[LMAP] for i in range(2)], 0)
    Wmix = Wmix.transpose(2, 0, 1, 3).copy()
    Wsc = np.stack([np.asarray(inputs["W_sc"][i], np.float32)[LMAP] for i in range(2)], 0)
    Wsc = Wsc.transpose(2, 0, 1, 3).copy()
    Wp = np.stack([np.asarray(inputs[f"Wp{j}"], np.float32) for j in (1, 2, 3)], 2)
    Wp = Wp.transpose(1, 0, 2, 3).copy()
    Wemb = np.asarray(inputs["W_embed"], np.float32)
    AE = np.asarray(inputs["atomic_energies"], np.float32)[:, None]
    Wro = np.concatenate([np.asarray(inputs["wE1"], np.float32)[:, None],
                          np.asarray(inputs["wD1"], np.float32)[:, None],
                          np.asarray(inputs["Wh"], np.float32),
                          np.asarray(inputs["wD2"], np.float32)[:, None]], 1)
    wE2 = np.asarray(inputs["wE2"], np.float32)[:, None]

    shared = dict(naT=naT, pos4=pos4, iota=iota, ident=ident,
                  nvec=nvec, R0s=R0s, R1s=R1s, R2s=R2s, R3e=R3e, Wmix=Wmix,
                  Wsc=Wsc, Wp=Wp, Wemb=Wemb, AE=AE, Wro=Wro, wE2=wE2)

    in_maps = []
    for k in range(NCORES):
        snd_pad = np.zeros(L_PAD, np.int64)
        rcv_glob = np.zeros(L_PAD, np.int64)
        rcv_loc = np.full(L_PAD, -1000.0, np.float32)
        for w in range(WPC):
            gw = k * WPC + w
            sel = win_id == gw
            cnt = int(counts[gw])
            base = w * W_CAP * CHUNK
            snd_pad[base:base + cnt] = snd_s[sel]
            rcv_glob[base:base + cnt] = rcv_s[sel]
            rcv_loc[base:base + cnt] = (rcv_s[sel] - gw * WIN).astype(np.float32)

        def wrap_idx(a):
            w16 = a.astype(np.int16).reshape(IDX_COLS, 16).T
            return np.tile(w16, (8, 1)).copy()

        own = slice(k * NPC, (k + 1) * NPC)
        m = dict(shared)
        m["gsnd"] = wrap_idx(snd_pad)
        m["grcv"] = wrap_idx(rcv_glob)
        m["rcvloc"] = np.ascontiguousarray(rcv_loc.reshape(NCHUNKS, CHUNK).T)
        m["naTo"] = np.ascontiguousarray(naT[:, own])
        goh = np.zeros((NPC, G), np.float32)
        goh[np.arange(NPC), batch[own]] = 1.0
        m["goh"] = np.ascontiguousarray(goh.reshape(WPC, 128, G).transpose(1, 0, 2))
        m["qown"] = np.ascontiguousarray(charges[own].reshape(WPC, 128).T)
        m["posown"] = np.ascontiguousarray(
            positions[own].reshape(WPC, 128, 3).transpose(1, 0, 2))
        in_maps.append(m)
    return in_maps, {}


# tensors loaded whole into SBUF constants pool
CONST_SPECS = dict(
    iota=([128, 128], f32), ident=([128, 128], f32), nvec=([128, NB], f32),
    R0s=([NB, 2, 64], f32), R1s=([64, 2, 64], f32), R2s=([64, 2, 64], f32),
    R3e=([64, 2, MC], f32), Wmix=([C, 2, NSH, C], f32), Wsc=([C, 2, NSH, C], f32),
    Wp=([Z, 2, 3, C], f32), Wemb=([Z, C], f32), AE=([Z, 1], f32),
    Wro=([C, 19], f32), wE2=([16, 1], f32),
    gsnd=([128, IDX_COLS], i16), grcv=([128, IDX_COLS], i16),
    rcvloc=([128, NCHUNKS], f32), naTo=([Z, NPC], f32),
    goh=([128, WPC, G], f32), qown=([128, WPC], f32), posown=([128, WPC, 3], f32),
)
DRAM_ONLY_SPECS = dict(naT=([Z, N], f32), pos4=([N, 4], f32))
INPUT_SPECS = {**CONST_SPECS, **DRAM_ONLY_SPECS}


def build_nc(num_devices=NCORES, sim_safe=False, use_f32r=True, phases=99, repeat=1):
    nc = bacc.Bacc("TRN2", target_bir_lowering=False, debug=False,
                   num_devices=num_devices, num_swdge_queues=NQ)
    inp = {name: nc.dram_tensor(name, shape, dt, kind="ExternalInput")
           for name, (shape, dt) in INPUT_SPECS.items()}
    y_out = nc.dram_tensor("y", [G, 4], f32, kind="ExternalOutput")
    table = nc.dram_tensor("table", [N, TROW], f32, kind="Internal")
    oh_dram = nc.dram_tensor("ohd", [NCHUNKS, 128, 128], bf16, kind="Internal")
    agin = nc.dram_tensor("agin", [NPC, C], f32, kind="Internal")
    agout = nc.dram_tensor("agout", [N, C], f32, kind="Internal",
                           addr_space="Shared")

    def silu(out_ap, in_ap, pool, tag="siltmp"):
        if not sim_safe:
            nc.scalar.activation(out_ap, in_ap, AF.Silu)
        else:
            sg = pool.tile(list(out_ap.shape), f32, tag=tag)
            nc.scalar.activation(sg[:], in_ap, AF.Sigmoid)
            nc.vector.tensor_tensor(out_ap, in_ap, sg[:], ALU.mult)

    def mmdt(ap):
        return ap.bitcast(f32r) if use_f32r else ap

    NQC = NCHUNKS // NQ
    IQC = IDX_COLS // NQ

    GNI = 768                       # idxs per dma_gather call (SWDGE ring limit)
    GCH = GNI // 128                # chunks per call
    GCALLS = L_PAD // NQ // GNI     # calls per queue

    def gather(dst_tile, src_dram, idx_tile, col0=0, ncol=TROW):
        for c in range(GCALLS):
            for q in range(NQ):
                b = q * NQC + c * GCH
                nc.gpsimd.dma_gather(
                    out_ap=dst_tile[:, b:b + GCH, :],
                    in_ap=src_dram.ap()[:, col0:col0 + ncol],
                    idxs_ap=idx_tile[:, b * 8:(b + GCH) * 8],
                    num_idxs=GNI, num_idxs_reg=GNI,
                    elem_size=ncol, elem_step=TROW, queue_num=q)

    with tile.TileContext(nc) as tc:
        with tc.tile_pool(name="const", bufs=1) as cst, \
             tc.tile_pool(name="big", bufs=1) as big, \
             tc.tile_pool(name="pmlp", bufs=2, space="PSUM") as pmlp, \
             tc.tile_pool(name="px", bufs=2, space="PSUM") as pxp, \
             tc.tile_pool(name="pa", bufs=2, space="PSUM") as pap, \
             tc.tile_pool(name="pmisc", bufs=2, space="PSUM") as pms:

            sb = {}
            for name, (shape, dt) in CONST_SPECS.items():
                t = cst.tile(shape, dt, tag=f"c_{name}")
                nc.sync.dma_start(out=t[:], in_=inp[name].ap())
                sb[name] = t
            # bf16 weight copies
            R0b = cst.tile([NB, 2, 64], bf16, tag="R0b")
            nc.scalar.activation(R0b[:], sb["R0s"][:], AF.Copy)
            Wscb = cst.tile([C, 2, NSH, C], bf16, tag="Wscb")
            nc.scalar.activation(Wscb[:], sb["Wsc"][:], AF.Copy)
            Wrob = cst.tile([C, 19], bf16, tag="Wrob")
            nc.scalar.activation(Wrob[:], sb["Wro"][:], AF.Copy)
            wE2b = cst.tile([16, 1], bf16, tag="wE2b")
            nc.scalar.activation(wE2b[:], sb["wE2"][:], AF.Copy)
            identb = cst.tile([128, 128], bf16, tag="identb")
            nc.scalar.activation(identb[:], sb["ident"][:], AF.Copy)
            R1r = cst.tile([64, 2, 64], f32r, tag="R1r")
            nc.scalar.activation(R1r[:], sb["R1s"][:], AF.Copy)
            R2r = cst.tile([64, 2, 64], f32r, tag="R2r")
            nc.scalar.activation(R2r[:], sb["R2s"][:], AF.Copy)
            R3er = cst.tile([64, 2, MC], f32r, tag="R3er")
            nc.scalar.activation(R3er[:], sb["R3e"][:], AF.Copy)

            # persistent tiles
            gs = big.tile([128, NCHUNKS, TROW], f32, tag="gs")
            gs32 = big.tile([128, NCHUNKS, 32], f32, tag="gs32")
            Ysb = big.tile([128, NCHUNKS, NSH], f32, tag="Y")
            rbT = big.tile([NB, L_PAD], bf16, tag="rbT")
            hT = big.tile([C, WPC, NSH * 128], bf16, tag="hT")
            h0oT = big.tile([C, WPC, 128], bf16, tag="h0oT")
            w123 = big.tile([128, WPC, 2, 3 * C], f32, tag="w123")
            e0_sb = big.tile([128, WPC], f32, tag="e0")
            vals = big.tile([128, WPC, 4], f32, tag="vals")

            # ---- setup phase (scratch scope)
            for _rep in range(repeat):
              with tc.tile_pool(name="setup", bufs=2) as st:
                  # h0 for all nodes -> table rows (h slot)
                  nat = st.tile([Z, N], f32, tag="nat", bufs=1)
                  nc.sync.dma_start(out=nat[:], in_=inp["naT"].ap())
                  for g8 in range(8):
                      h0g = st.tile([128, 8, C], f32, tag="h0g")
                      for i8 in range(8):
                          c64 = g8 * 8 + i8
                          ph = pms.tile([128, C], f32, tag="pm")
                          nc.tensor.matmul(ph[:], nat[:, c64 * 128:(c64 + 1) * 128],
                                           sb["Wemb"][:], start=True, stop=True)
                          nc.scalar.activation(h0g[:, i8, :], ph[:], AF.Copy)
                      nc.sync.dma_start(
                          out=table.ap()[g8 * 1024:(g8 + 1) * 1024, :]
                              .rearrange("(c p) r -> p c r", p=128)[:, :, 4:4 + C],
                          in_=h0g[:])
                  nc.sync.dma_start(out=table.ap()[:, 0:4], in_=inp["pos4"].ap())

                  # own-node quantities
                  h0o = st.tile([128, WPC, C], f32, tag="h0o")
                  for w in range(WPC):
                      nao = sb["naTo"][:, w * 128:(w + 1) * 128]
                      ph = pms.tile([128, C], f32, tag="pm")
                      nc.tensor.matmul(ph[:], nao, sb["Wemb"][:], start=True, stop=True)
                      nc.scalar.activation(h0o[:, w, :], ph[:], AF.Copy)
                      pe0 = pms.tile([128, 1], f32, tag="pm")
                      nc.tensor.matmul(pe0[:], nao, sb["AE"][:], start=True, stop=True)
                      nc.scalar.activation(e0_sb[:, w].unsqueeze(1), pe0[:], AF.Copy)
                      pw = pms.tile([128, 2 * 3 * C], f32, tag="pm")
                      for i in range(2):
                          for j in range(3):
                              nc.tensor.matmul(pw[:, (i * 3 + j) * C:(i * 3 + j + 1) * C],
                                               nao, sb["Wp"][:, i, j, :],
                                               start=True, stop=True)
                      nc.scalar.activation(
                          w123[:, w, :, :].rearrange("p a b -> p (a b)"), pw[:], AF.Copy)
                      pt = pms.tile([C, 128], f32, tag="pm")
                      nc.tensor.transpose(pt[:], h0o[:, w, :], sb["ident"][:])
                      nc.scalar.activation(h0oT[:, w, :], pt[:], AF.Copy)

                  # one-hot blocks -> DRAM (one DMA per window)
                  for w in range(WPC):
                      ohb = st.tile([128, W_CAP, 128], bf16, tag="ohb")
                      for j in range(W_CAP):
                          nc.vector.tensor_scalar(ohb[:, j, :], sb["iota"][:],
                                                  sb["rcvloc"][:, w * W_CAP + j].unsqueeze(1),
                                                  1.0 / AVG_NEIGH, ALU.is_equal, ALU.mult)
                      nc.sync.dma_start(
                          out=oh_dram.ap()[w * W_CAP:(w + 1) * W_CAP]
                              .rearrange("j p c -> p j c"),
                          in_=ohb[:])

              # ---- gathers + geometry (scratch scope)
              if phases >= 2:
                with tc.tile_pool(name="geos", bufs=1) as gsc:
                    gr = gsc.tile([128, NCHUNKS, 32], f32, tag="gr")
                    gather(gs, table, sb["gsnd"])
                    gather(gr, table, sb["grcv"], col0=0, ncol=32)

                    geo = gsc.tile([128, NCHUNKS, 14], f32, tag="geo")
                    vec, sq = geo[:, :, 0:3], geo[:, :, 3:6]
                    r2, r_, rinv = geo[:, :, 6], geo[:, :, 7], geo[:, :, 8]
                    u = geo[:, :, 9:12]
                    t0, t1 = geo[:, :, 12], geo[:, :, 13]
                    BC = [128, NCHUNKS, 3]
                    nc.vector.tensor_tensor(vec, gr[:, :, 0:3], gs[:, :, 0:3], ALU.subtract)
                    nc.vector.tensor_tensor(sq, vec, vec, ALU.mult)
                    nc.vector.tensor_reduce(r2.unsqueeze(2), sq, mybir.AxisListType.X, ALU.add)
                    nc.vector.tensor_scalar_add(r2.unsqueeze(2), r2.unsqueeze(2), 1e-12)
                    nc.scalar.activation(r_.unsqueeze(2), r2.unsqueeze(2), AF.Sqrt)
                    nc.vector.reciprocal(rinv.unsqueeze(2), r_.unsqueeze(2))
                    nc.vector.tensor_tensor(u, vec, rinv.unsqueeze(2).broadcast_to(BC),
                                            ALU.mult)
                    ux = u[:, :, 0].unsqueeze(2)
                    uy = u[:, :, 1].unsqueeze(2)
                    uz = u[:, :, 2].unsqueeze(2)
                    nc.vector.memset(Ysb[:, :, 0].unsqueeze(2), 1.0)
                    nc.scalar.activation(Ysb[:, :, 1:4], u, AF.Copy, scale=S3)
                    nc.vector.scalar_tensor_tensor(Ysb[:, :, 4].unsqueeze(2), ux, S15, uy,
                                                   ALU.mult, ALU.mult)
                    nc.vector.scalar_tensor_tensor(Ysb[:, :, 5].unsqueeze(2), uy, S15, uz,
                                                   ALU.mult, ALU.mult)
                    nc.vector.tensor_tensor(t0.unsqueeze(2), uz, uz, ALU.mult)
                    nc.scalar.activation(Ysb[:, :, 6].unsqueeze(2), t0.unsqueeze(2), AF.Copy,
                                         scale=3.0 * S5 / 2.0, bias=-S5 / 2.0)
                    nc.vector.scalar_tensor_tensor(Ysb[:, :, 7].unsqueeze(2), ux, S15, uz,
                                                   ALU.mult, ALU.mult)
                    nc.vector.tensor_tensor(t0.unsqueeze(2), ux, uy, ALU.add)
                    nc.vector.tensor_tensor(t1.unsqueeze(2), ux, uy, ALU.subtract)
                    nc.vector.scalar_tensor_tensor(Ysb[:, :, 8].unsqueeze(2),
                                                   t0.unsqueeze(2), S15 / 2.0,
                                                   t1.unsqueeze(2), ALU.mult, ALU.mult)
                  # radial basis
                    rbw = gsc.tile([128, NCHUNKS, NB], f32, tag="rbw")
                    BC8 = [128, NCHUNKS, NB]
                    nc.vector.tensor_tensor(rbw[:], r_.unsqueeze(2).broadcast_to(BC8),
                                            sb["nvec"].unsqueeze(1).broadcast_to(BC8),
                                            ALU.mult)
                  # range-reduce arg to [-pi, pi]: t = arg - 2pi*int(arg/2pi),
                  # then shift if > pi (robust to trunc or round-to-nearest casts)
                    rmsk = gsc.tile([128, NCHUNKS, NB], f32, tag="rmsk")
                    rki = gsc.tile([128, NCHUNKS, NB], mybir.dt.int32, tag="rki")
                    nc.vector.tensor_scalar(rmsk[:], rbw[:], 1.0 / TWO_PI, None, ALU.mult)
                    nc.vector.tensor_copy(rki[:], rmsk[:])
                    nc.vector.tensor_copy(rmsk[:], rki[:])
                    nc.vector.scalar_tensor_tensor(rbw[:], rmsk[:], -TWO_PI, rbw[:],
                                                   ALU.mult, ALU.add)
                    nc.vector.tensor_scalar(rmsk[:], rbw[:], math.pi, None, ALU.is_gt)
                    nc.vector.scalar_tensor_tensor(rbw[:], rmsk[:], -TWO_PI, rbw[:],
                                                   ALU.mult, ALU.add)
                    nc.vector.tensor_scalar(rbw[:], rbw[:], math.pi, None, ALU.min)
                    nc.vector.tensor_scalar(rbw[:], rbw[:], -math.pi, None, ALU.max)
                    nc.scalar.activation(rbw[:], rbw[:], AF.Sin)
                  # envelope: xx=t0; x2=t1; x4=sq0; x5=sq1; q1=sq2; q2=t1; env=r2
                    xx = t0.unsqueeze(2)
                    nc.vector.tensor_scalar(xx, r_.unsqueeze(2), 1.0 / R_MAX, None, ALU.mult)
                    x2 = t1.unsqueeze(2)
                    nc.vector.tensor_tensor(x2, xx, xx, ALU.mult)
                    x4 = geo[:, :, 3].unsqueeze(2)
                    nc.vector.tensor_tensor(x4, x2, x2, ALU.mult)
                    x5 = geo[:, :, 4].unsqueeze(2)
                    nc.vector.tensor_tensor(x5, x4, xx, ALU.mult)
                    q1 = geo[:, :, 5].unsqueeze(2)
                    nc.scalar.activation(q1, xx, AF.Copy, scale=ENV_C, bias=ENV_B)
                    q2 = t1.unsqueeze(2)
                    nc.vector.tensor_tensor(q2, q1, xx, ALU.mult)
                    nc.vector.tensor_scalar_add(q2, q2, ENV_A)
                    env = r2.unsqueeze(2)
                    nc.vector.tensor_tensor(env, x5, q2, ALU.mult)
                    nc.vector.tensor_scalar_add(env, env, 1.0)
                    mlt = geo[:, :, 3].unsqueeze(2)
                    nc.vector.tensor_scalar(mlt, xx, 1.0, None, ALU.is_lt)
                    nc.vector.tensor_tensor(env, env, mlt, ALU.mult)
                    wfac = geo[:, :, 4].unsqueeze(2)
                    nc.vector.scalar_tensor_tensor(wfac, rinv.unsqueeze(2), PREF, env,
                                                   ALU.mult, ALU.mult)
                    nc.vector.tensor_tensor(rbw[:], rbw[:],
                                            wfac.broadcast_to(BC8), ALU.mult)
                    # transpose rb -> rbT (bf16 out), f32 PE transposes
                    for t4 in range(NCHUNKS // 4):
                        ptr = pms.tile([NB, 512], f32, tag="pm")
                        for j in range(4):
                            g = t4 * 4 + j
                            nc.tensor.transpose(ptr[:, j * 128:(j + 1) * 128],
                                                rbw[:, g, :], sb["ident"][:])
                        nc.scalar.activation(rbT[:, t4 * 512:(t4 + 1) * 512], ptr[:], AF.Copy)

              # ---- iterations
              with tc.tile_pool(name="wk", bufs=3) as wk, \
                   tc.tile_pool(name="nd", bufs=2) as ndp:
                  nc.vector.memset(vals[:], 0.0)
                  for it in range(2 if phases >= 4 else (1 if phases >= 3 else 0)):
                      for w in range(WPC):
                          ohw = wk.tile([128, W_CAP, 128], bf16, tag="ohw")
                          nc.sync.dma_start(
                              out=ohw[:],
                              in_=oh_dram.ap()[w * W_CAP:(w + 1) * W_CAP]
                                  .rearrange("j p c -> p j c"))
                          pA = pap.tile([128, MC], f32, tag="pA")
                          for s6 in range(6):
                              sci = w * 6 + s6
                              ee = sci * SC
                              p1 = pmlp.tile([64, SC], f32, tag="pmlp")
                              nc.tensor.matmul(p1[:], R0b[:, it, :],
                                               rbT[:, ee:ee + SC], start=True, stop=True)
                              s1 = wk.tile([64, SC], f32r, tag="s1")
                              silu(s1[:], p1[:], wk)
                              p2 = pmlp.tile([64, SC], f32, tag="pmlp")
                              nc.tensor.matmul(p2[:], R1r[:, it, :],
                                               s1[:], start=True, stop=True)
                              s2 = wk.tile([64, SC], f32r, tag="s2")
                              silu(s2[:], p2[:], wk)
                              p3 = pmlp.tile([64, SC], f32, tag="pmlp")
                              nc.tensor.matmul(p3[:], R2r[:, it, :],
                                               s2[:], start=True, stop=True)
                              s3 = wk.tile([64, SC], f32r, tag="s3")
                              silu(s3[:], p3[:], wk)
                              g3 = sci * 3
                              yh3 = wk.tile([128, 3, MC], f32, tag="yh3")
                              nc.vector.tensor_tensor(
                                  yh3[:].rearrange("p t (m c) -> p t m c", m=NSH),
                                  Ysb[:, g3:g3 + 3, :].unsqueeze(3)
                                      .broadcast_to([128, 3, NSH, C]),
                                  gs[:, g3:g3 + 3, 4:4 + C].unsqueeze(2)
                                      .broadcast_to([128, 3, NSH, C]),
                                  ALU.mult)
                              for j in range(3):
                                  g = sci * 3 + j
                                  px = pxp.tile([128, MC], f32, tag="px")
                                  nc.tensor.matmul(px[:], s3[:, j * 128:(j + 1) * 128],
                                                   R3er[:, it, :],
                                                   start=True, stop=True)
                                  msg = wk.tile([128, MC], bf16, tag="msg")
                                  nc.vector.tensor_tensor(msg[:], yh3[:, j, :], px[:],
                                                          ALU.mult)
                                  nc.tensor.matmul(pA[:], ohw[:, s6 * 3 + j, :], msg[:],
                                                   start=(s6 == 0 and j == 0),
                                                   stop=(s6 == 5 and j == 2))

                          # node phase
                          A_sb = ndp.tile([128, MC], f32, tag="Asb")
                          nc.scalar.activation(A_sb[:], pA[:], AF.Copy)
                          AT = ndp.tile([C, NSH * 128], f32, tag="AT")
                          for t4 in range(3):
                              hi = min(4, NSH - t4 * 4)
                              ptA = pms.tile([C, 512], f32, tag="pm")
                              for j in range(hi):
                                  m = t4 * 4 + j
                                  nc.tensor.transpose(ptA[:, j * 128:(j + 1) * 128],
                                                      A_sb[:, m * C:(m + 1) * C],
                                                      sb["ident"][:])
                              nc.scalar.activation(AT[:, t4 * 512:t4 * 512 + hi * 128],
                                                   ptA[:, 0:hi * 128], AF.Copy)
                          pA2 = pms.tile([128, MC], f32, tag="pm")
                          for m in range(NSH):
                              nc.tensor.matmul(pA2[:, m * C:(m + 1) * C],
                                               AT[:, m * 128:(m + 1) * 128],
                                               sb["Wmix"][:, it, m, :],
                                               start=True, stop=True)
                          psc = pms.tile([128, MC], f32, tag="pm")
                          if it == 0:
                              nc.tensor.matmul(psc[:, 0:C], h0oT[:, w, :],
                                               Wscb[:, 0, 0, :], start=True, stop=True)
                              sc_sb = ndp.tile([128, C], f32, tag="scsb")
                              nc.scalar.activation(sc_sb[:], psc[:, 0:C], AF.Copy)
                          else:
                              for m in range(NSH):
                                  nc.tensor.matmul(psc[:, m * C:(m + 1) * C],
                                                   hT[:, w, m * 128:(m + 1) * 128], Wscb[:, 1, m, :],
                                                   start=True, stop=True)
                              sc_sb = ndp.tile([128, MC], f32, tag="scsb9")
                              nc.scalar.activation(sc_sb[:], psc[:], AF.Copy)
                          wslc = w123[:, w, it, :]
                          F = ndp.tile([128, C], f32, tag="F")
                          nc.vector.tensor_tensor(F[:], wslc[:, 2 * C:3 * C],
                                                  pA2[:, 0:C], ALU.mult)
                          nc.vector.tensor_tensor(F[:], F[:], wslc[:, C:2 * C], ALU.add)
                          nc.vector.tensor_tensor(F[:], F[:], pA2[:, 0:C], ALU.mult)
                          nc.vector.tensor_tensor(F[:], F[:], wslc[:, 0:C], ALU.add)
                          hw_t = ndp.tile([128, MC], f32, tag="hw")
                          nc.vector.tensor_tensor(
                              hw_t[:].rearrange("p (m c) -> p m c", m=NSH),
                              pA2[:].rearrange("p (m c) -> p m c", m=NSH),
                              F[:].unsqueeze(1).broadcast_to([128, NSH, C]), ALU.mult)
                          if it == 0:
                              nc.vector.tensor_tensor(hw_t[:, 0:C], hw_t[:, 0:C],
                                                      sc_sb[:], ALU.add)
                          else:
                              nc.vector.tensor_tensor(hw_t[:], hw_t[:], sc_sb[:], ALU.add)
                          n_m = NSH if it == 0 else 4
                          for t4 in range((n_m + 3) // 4):
                              hi = min(4, n_m - t4 * 4)
                              pth = pms.tile([C, 512], f32, tag="pm")
                              for j in range(hi):
                                  m = t4 * 4 + j
                                  nc.tensor.transpose(pth[:, j * 128:(j + 1) * 128],
                                                      hw_t[:, m * C:(m + 1) * C],
                                                      sb["ident"][:])
                              nc.scalar.activation(
                                  hT[:, w, t4 * 512:t4 * 512 + hi * 128],
                                  pth[:, 0:hi * 128], AF.Copy)
                          if it == 0:
                              nc.sync.dma_start(
                                  out=agin.ap()[w * 128:(w + 1) * 128, :],
                                  in_=hw_t[:, 0:C])
                              prd = pms.tile([128, 4], f32, tag="pm")
                              nc.tensor.matmul(prd[:, 0:1], hT[:, w, 0:128], Wrob[:, 0:1],
                                               start=True, stop=True)
                              for m in (1, 2, 3):
                                  nc.tensor.matmul(prd[:, m:m + 1], hT[:, w, m * 128:(m + 1) * 128],
                                                   Wrob[:, 1:2], start=True, stop=True)
                              nc.vector.scalar_tensor_tensor(
                                  vals[:, w, 0].unsqueeze(1), prd[:, 0:1], 1.0,
                                  e0_sb[:, w].unsqueeze(1), ALU.mult, ALU.add)
                              nc.scalar.activation(vals[:, w, 1:4], prd[:, 1:4], AF.Copy)
                          else:
                              phid = pms.tile([128, 16], f32, tag="pm")
                              nc.tensor.matmul(phid[:], hT[:, w, 0:128], Wrob[:, 2:18],
                                               start=True, stop=True)
                              hid = ndp.tile([128, 16], f32, tag="hid")
                              silu(hid[:], phid[:], ndp)
                              pht = pms.tile([16, 128], f32, tag="pm")
                              nc.tensor.transpose(pht[:], hid[:], sb["ident"][:])
                              hidT = ndp.tile([16, 128], bf16, tag="hidT")
                              nc.scalar.activation(hidT[:], pht[:], AF.Copy)
                              prd = pms.tile([128, 4], f32, tag="pm")
                              nc.tensor.matmul(prd[:, 0:1], hidT[:], wE2b[:],
                                               start=True, stop=True)
                              for m in (1, 2, 3):
                                  nc.tensor.matmul(prd[:, m:m + 1], hT[:, w, m * 128:(m + 1) * 128],
                                                   Wrob[:, 18:19], start=True, stop=True)
                              nc.vector.tensor_tensor(vals[:, w, :], vals[:, w, :],
                                                      prd[:], ALU.add)

                      if it == 0:
                          if num_devices > 1:
                              nc.gpsimd.collective_compute(
                                  "AllGather", ALU.bypass,
                                  replica_groups=[list(range(num_devices))],
                                  ins=[agin.ap()], outs=[agout.ap()])
                              nc.sync.dma_start(
                                  out=table.ap().rearrange("(c p) r -> p c r", p=128)[:, :, 4:4 + C],
                                  in_=agout.ap().rearrange("(c p) k -> p c k", p=128))
                          else:
                              nc.sync.dma_start(
                                  out=table.ap()[0:NPC, :]
                                      .rearrange("(w p) r -> p w r", p=128)[:, :, 4:4 + C],
                                  in_=agin.ap().rearrange("(w p) c -> p w c", p=128))
                          gather(gs32, table, sb["gsnd"], col0=4, ncol=32)

                  # final reduction
                  pO = pms.tile([G, 4], f32, tag="pm")
                  for w in range(WPC):
                      nc.vector.scalar_tensor_tensor(
                          vals[:, w, 1:4], sb["posown"][:, w, :],
                          sb["qown"][:, w].unsqueeze(1), vals[:, w, 1:4],
                          ALU.mult, ALU.add)
                  for w in range(WPC):
                      nc.tensor.matmul(pO[:], sb["goh"][:, w, :], vals[:, w, :],
                                       start=(w == 0), stop=(w == WPC - 1))
                  y_sb = ndp.tile([G, 4], f32, tag="ysb")
                  nc.scalar.activation(y_sb[:], pO[:], AF.Copy)
                  nc.sync.dma_start(out=y_out.ap(), in_=y_sb[:])

    nc.compile()
    return nc


# ----------------------------------------------------------------------------
# Public entry point: full inputs -> full output. Shards internally across
# 8 NeuronCores, compiles once (cached), executes via run_bass_kernel_spmd.
# ----------------------------------------------------------------------------
from concourse.bass_utils import run_bass_kernel_spmd as _run_spmd

_NC_CACHE = {}


def _get_nc():
    if "nc" not in _NC_CACHE:
        _NC_CACHE["nc"] = build_nc(num_devices=NCORES, sim_safe=False,
                                   use_f32r=True)
    return _NC_CACHE["nc"]


def kernel(**inputs):
    np_inputs = {k: np.asarray(v) for k, v in inputs.items()}
    in_maps, _ = host_prep(np_inputs)
    nc = _get_nc()
    res = _run_spmd(nc, in_maps, core_ids=list(range(NCORES)))
    y = sum(np.asarray(res.results[k]["y"], dtype=np.float64)
            for k in range(NCORES))
    return y.astype(np.float32)

